# revision 70
# baseline (speedup 1.0000x reference)
"""Bass/Trainium2 kernel for masked (padding) multi-head self-attention.

Problem: B=2, T=2048, C=1024, H=16 heads of DH=64.
  q/k/v = x @ W* + b*  ->  att = softmax(mask(q k^T / 8))  ->  y = att @ v

Sharding over 8 NeuronCores: core = (batch b, head-group hg) with
b = core // 4, hg = core % 4; each core computes 4 heads for one batch
element (its [T, 256] slice of q/k/v from the Wq/Wk/Wv column slice).

Pipeline (per core):
  - Host gathers valid tokens (mask!=0), pads to TP = roundup(max_tv, 16)
    (1040 for the seed-0 mask vs 2048 raw; ragged last k-tile of
    ML = TP-1024 rows), transposes x, converts x^T/Wq/Wk/Wv to fp16 and
    pre-swizzles them into the exact SBUF layouts (full-rate DMA runs).
  - All matmul operands are fp16 (1 PE cycle/row at any free size, vs
    f32r needing >=256; ~0.05% quantization so softmax logit noise stays
    ~1e-3 -- fp8 DoubleRow was tried and FAILS the 2e-2 gate: its ~3%
    logit noise gives 5e-2 errors on near-tied attention rows, and
    e^(s-2) overflows e4m3 at the seed-0 max score).  PSUM stays fp32.
  - Phase A (under the x^T DMA, c-tile-major): q/k d0 projections for
    the PSUM-bank-aligned q-chunks [512, 512, TP-1024] minus the middle
    chunk; kT-j0 evacuates on ACT parallel to qT on DVE, so tile 0's
    scores run j-split [j0, j2 | j1] and the first exp fires ~11us.
  - Phase 1: h0/h1 scores s^T = k^T q per k-tile; ONE exp per tile reads
    the flat [128, TP] PSUM span into an fp16 e-tile (bias -2 folded in,
    cancels in the softmax ratio).  Remaining projections (q/k d1, the
    d0 middle chunk, v) drip-feed as ~0.85us filler units under the
    ACT-paced stream.
  - Phase 2: h2/h3 scores (ragged k-tile first) interleaved with AV
    pieces: per (head, chunk) accumulation groups over k-tiles, with an
    extra ones column (M=65) making output row 64 the softmax
    denominator.  h0/h1 run one 9-matmul round; h2/h3 three
    release-gated rounds whose small last round folds the previous acc
    via an identity matmul so its evacuation is a copy split across
    ACT/DVE, followed by one whole-head out-DMA.  Host divides
    numerator/denominator and scatters.

Cost-model timeline ~68.8us (vs 77.4us baseline): PE-bound, busy ~54us
(qkT 13.8 + v 7.7 + scores 15.6 + AV 15.6 + overheads), ACT exp stream
dense 12.7->54us, ~3.5us evac+DMA tail.  HW-verified rel err 8.9e-4.
"""

import sys
from collections import deque

sys.path.insert(0, "/opt/trn_rl_repo")

import ml_dtypes
import numpy as np

import concourse.bacc as bacc
import concourse.mybir as mybir
import concourse.tile as tile
from concourse import bass_utils

F32 = mybir.dt.float32
F32R = mybir.dt.float32r
FP16 = mybir.dt.float16
AF = mybir.ActivationFunctionType
NPFP16 = np.float16

B, T, C, H = 2, 2048, 1024, 16
DH = C // H            # 64
HPC = 4                # heads per core
CSL = HPC * DH         # 256, per-core column slice of C
N_CORES = 8
NCT = C // 128         # 8 contraction tiles over C
VW = 80                # padded v row width (DoubleRow needs stride%16==0)
EB = -2.0              # exp bias shift, cancels in softmax ratio

_CACHE: dict = {}


def _pick_dims(max_valid: int):
    """TP (multiple of 2 for fp16 DMA runs, >=128), k-tiles, last rows."""
    tp = max(-(-max_valid // 2) * 2, 128)
    nkt = -(-tp // 128)
    ml = tp - 128 * (nkt - 1)
    return tp, nkt, ml


def _chunks(tp: int):
    """PSUM-bank-aligned q-chunks: [512, 512, ...] + ragged tail."""
    return [(o, min(512, tp - o)) for o in range(0, tp, 512)]


def _build(tp: int, nkt: int, ml: int, with_bv: bool = False):
    nc = bacc.Bacc("TRN2", target_bir_lowering=False, debug=False,
                   num_devices=N_CORES)

    chunks = _chunks(tp)
    nch = len(chunks)

    # phase-2 processing order: the ragged k-tile first, so the last
    # h2/h3 AV round (the tail after the final exp) covers full tiles
    proc_t = [nkt - 1] + list(range(nkt - 1)) if nkt > 1 else [0]
    # h0/h1 AV: one accumulation round over all k-tiles (everything is
    # ready when phase 2 starts).  h2/h3: three release-gated rounds,
    # the last one small, in proc_t position space [a, b).
    if nkt >= 5:
        rounds23 = [(0, 3), (3, nkt - 2), (nkt - 2, nkt)]
    else:
        rounds23 = [(0, nkt)]

    # host-pre-swizzled inputs (partition-first layouts)
    xt_d = nc.dram_tensor("xt", [128, NCT, tp], FP16, kind="ExternalInput")
    wq_d = nc.dram_tensor("wq", [128, 2, NCT, 128], FP16, kind="ExternalInput")
    wk_d = nc.dram_tensor("wk", [128, 2, NCT, 128], FP16, kind="ExternalInput")
    wv_d = nc.dram_tensor("wv", [128, NCT, CSL], FP16, kind="ExternalInput")
    # bias128: col 0..3 = bqk (bq d0, bq d1, bk d0, bk d1), col 4.. = ebias
    bias128_d = nc.dram_tensor("bias128", [128, 4 + nkt], F32,
                               kind="ExternalInput")
    onesv_d = nc.dram_tensor("onesv", [128, nkt * HPC], FP16,
                             kind="ExternalInput")
    ident_d = nc.dram_tensor("ident", [DH + 1, DH + 1], F32,
                             kind="ExternalInput")
    if with_bv:
        misc1_d = nc.dram_tensor("misc1", [1, CSL + 128], FP16,
                                 kind="ExternalInput")
    out_d = nc.dram_tensor("out", [DH + 1, HPC, tp], F32,
                           kind="ExternalOutput")

    def mrows(t):
        return ml if t == nkt - 1 else 128

    with tile.TileContext(nc) as tc:
        with tc.tile_pool(name="const", bufs=1) as cp:
            xt_sb = cp.tile([128, NCT, tp], FP16, tag="xt")
            wq_sb = cp.tile([128, 2, NCT, 128], FP16, tag="wq")
            wk_sb = cp.tile([128, 2, NCT, 128], FP16, tag="wk")
            wv_sb = cp.tile([128, NCT, CSL], FP16, tag="wv")
            bias128_sb = cp.tile([128, 4 + nkt], F32, tag="bias128")
            qt_sb = cp.tile([128, 2, tp], FP16, tag="qt")
            kt_sb = cp.tile([128, 2, tp], FP16, tag="kt")
            v_sb = cp.tile([128, nkt, HPC, VW], FP16, tag="v")
            acc_sb = cp.tile([DH + 1, HPC, tp], F32R, tag="acc")
            ident_sb = cp.tile([DH + 1, DH + 1], F32R,
                               tag="ident")
            bqk_sb = bias128_sb[:, 0:4]
            ebias_sb = bias128_sb[:, 4:4 + nkt]
            if with_bv:
                misc1_sb = cp.tile([1, CSL + 128], FP16, tag="misc1")
                bv_sb = misc1_sb[:, 0:CSL]
                ones_sb = misc1_sb[:, CSL:CSL + 128]

            scratch = cp.tile([1, 8], F32, tag="scratch")

            # critical-path DMA order: wq d0 (full-rate thanks to the host
            # swizzle), bias, then x^T c-tiles with wk d0 slotted after c0;
            # everything else after x^T completes.
            nc.sync.dma_start(wq_sb[:, 0], wq_d.ap()[:, 0])
            nc.sync.dma_start(bias128_sb[:], bias128_d.ap()[:])
            nc.sync.dma_start(xt_sb[:, 0], xt_d.ap()[:, 0])
            nc.sync.dma_start(wk_sb[:, 0], wk_d.ap()[:, 0])
            for i in range(1, NCT):
                nc.sync.dma_start(xt_sb[:, i], xt_d.ap()[:, i])
            nc.sync.dma_start(wq_sb[:, 1], wq_d.ap()[:, 1])
            nc.sync.dma_start(wk_sb[:, 1], wk_d.ap()[:, 1])
            nc.sync.dma_start(wv_sb[:], wv_d.ap()[:])
            nc.sync.dma_start(
                v_sb[:, :, :, DH],
                onesv_d.ap().rearrange("p (t h) -> p t h", h=HPC))
            nc.sync.dma_start(
                ident_sb[:], ident_d.ap()[:].bitcast(F32R))
            if with_bv:
                nc.sync.dma_start(misc1_sb[:], misc1_d.ap()[:])

            # tiny PE-ramp dummy source memset first (dummies wait on
            # it), then the ACT exp table warm, all during the DMA window
            wsc = cp.tile([128, 16], FP16, tag="wsc")
            nc.gpsimd.memset(wsc[:], 0.0)
            nc.gpsimd.memset(scratch[:], 0.0)
            nc.scalar.activation(scratch[:], scratch[:], AF.Exp)

            def evac_qk(o_sb, d, off, w, ps, bcol, on_act=False):
                if on_act:
                    nc.scalar.activation(
                        o_sb[:, d, off:off + w], ps[:, 0:w], AF.Identity,
                        bias=bqk_sb[:, bcol + d:bcol + d + 1])
                else:
                    nc.vector.tensor_scalar_add(
                        o_sb[:, d, off:off + w], ps[:, 0:w],
                        bqk_sb[:, bcol + d:bcol + d + 1])

            # phase A, two passes.  Pass 1 (c-tile-major, pipelining with
            # the x^T DMA): the j0 + last-chunk d0 groups whose evacs gate
            # the first exps, plus q-d1-j0 as filler.  Pass 2 (PE backlog
            # right after): the q/k d0 j1 groups, q first — tile 0's exps
            # run j-split [j0, j2, then j1] so ACT starts ~3.5us earlier.
            # kT-j0 evacuates on ACT, everything else on DVE.
            p1_js = [0] + ([nch - 1] if nch > 2 else [])
            p2_js = [j for j in range(1, nch) if j not in p1_js]
            pa_p1 = [(wi, 0, j) for j in p1_js for wi in (0, 1)]
            pa_p2 = []
            with tc.tile_pool(name="pa", bufs=len(pa_p1) + len(pa_p2),
                              space="PSUM") as pa:
                # bridge the PE p-state ramp during the DMA head
                for _ in range(12):
                    wps = pa.tile([16, 16], F32, tag="a", name="wps")
                    nc.tensor.matmul(wps[:], wsc[:], wsc[:],
                                     start=True, stop=True)
                pga = {}
                for spec in pa_p1 + pa_p2:
                    pga[spec] = pa.tile([128, 512], F32, tag="a",
                                        name="pqk0")

                def pa_mm(spec, ct):
                    wi, d, j = spec
                    w_sb = wq_sb if wi == 0 else wk_sb
                    off, w = chunks[j]
                    nc.tensor.matmul(
                        pga[spec][:, 0:w],
                        w_sb[:, d, ct, :],
                        xt_sb[:, ct, off:off + w],
                        start=(ct == 0), stop=(ct == NCT - 1),
                    )

                for ct in range(NCT):
                    for spec in pa_p1:
                        pa_mm(spec, ct)
                evac_qk(kt_sb, 0, chunks[0][0], chunks[0][1],
                        pga[(1, 0, 0)], 2, on_act=True)
                for j in p1_js:
                    off, w = chunks[j]
                    evac_qk(qt_sb, 0, off, w, pga[(0, 0, j)], 0)
                for j in p1_js[1:]:
                    off, w = chunks[j]
                    evac_qk(kt_sb, 0, off, w, pga[(1, 0, j)], 2)


            ebufs = 4 * nkt + 2

            with (
                tc.tile_pool(name="ops", bufs=2, space="PSUM") as ops,
                tc.tile_pool(name="sps", bufs=2, space="PSUM") as sps_pool,
                tc.tile_pool(name="epool", bufs=ebufs) as ep,
            ):
                e_tiles: dict = {}

                def get_e(h, t):
                    if (h, t) not in e_tiles:
                        e_tiles[(h, t)] = ep.tile([128, tp], FP16,
                                                  tag="e", name="e")
                    return e_tiles[(h, t)]

                def qk_unit(u):
                    # 256-wide column sub-chunks (full contraction) so each
                    # filler unit is only ~0.85us of PE work
                    wi, o_sb, bcol, d, off, w = u
                    w_sb = wq_sb if wi == 0 else wk_sb
                    ps = ops.tile([128, 256], F32, tag="o", name="pqk1")
                    for ct in range(NCT):
                        nc.tensor.matmul(
                            ps[:, 0:w],
                            w_sb[:, d, ct, :],
                            xt_sb[:, ct, off:off + w],
                            start=(ct == 0), stop=(ct == NCT - 1),
                        )
                    evac_qk(o_sb, d, off, w, ps, bcol)

                def v_unit(t):
                    mt = mrows(t)
                    ps = ops.tile([128, CSL], F32, tag="o", name="pv")
                    for ct in range(NCT):
                        nc.tensor.matmul(
                            ps[0:mt, :],
                            xt_sb[:, ct, 128 * t:128 * t + mt],
                            wv_sb[:, ct, :],
                            start=(ct == 0),
                            stop=(not with_bv and ct == NCT - 1),
                        )
                    if with_bv:
                        nc.tensor.matmul(ps[0:mt, :], ones_sb[:, 0:mt],
                                         bv_sb[:], start=False, stop=True)
                    nc.vector.tensor_copy(
                        v_sb[0:mt, t, :, 0:DH],
                        ps[0:mt, :].rearrange("p (h d) -> p h d", h=HPC),
                    )

                def scores_pair(hA, hB, t, filler=None, jsplit=False):
                    # hA/hB share a qT/kT d-tile at partition offsets 0/64.
                    # Per-head emission: the head's chunk matmuls then its
                    # exp, so ACT is fed after only 3 matmuls; the filler
                    # hook runs after each head.  jsplit (tile 0): emit the
                    # pass-1 chunks' matmuls+exps for BOTH heads first, the
                    # pass-2 chunk (whose qT evacuates later) behind them.
                    mt = mrows(t)
                    pd = hA // 2

                    def smm(ps, h, j):
                        off, w = chunks[j]
                        po = (h % 2) * 64
                        nc.tensor.matmul(
                            ps[0:mt, j, 0:w],
                            kt_sb[po:po + 64, pd, 128 * t:128 * t + mt],
                            qt_sb[po:po + 64, pd, off:off + w],
                            start=True, stop=True,
                        )

                    def sexp(ps, e_t, js):
                        flat = ps.rearrange("p a b -> p (a b)")
                        if js is None:
                            nc.scalar.activation(
                                e_t[:, 0:tp], flat[:, 0:tp], AF.Exp,
                                bias=ebias_sb[:, t:t + 1], scale=0.125,
                            )
                            return
                        for j in js:
                            off, w = chunks[j]
                            nc.scalar.activation(
                                e_t[:, off:off + w],
                                flat[:, off:off + w], AF.Exp,
                                bias=ebias_sb[:, t:t + 1], scale=0.125,
                            )

                    if jsplit and nch > 1:
                        st = {}
                        for h in (hA, hB):
                            st[h] = (sps_pool.tile([128, nch, 512], F32,
                                                   tag="s", name="sps"),
                                     get_e(h, t))
                        for h in (hA, hB):
                            ps, e_t = st[h]
                            for j in p1_js:
                                smm(ps, h, j)
                            sexp(ps, e_t, p1_js)

                        def finish():
                            for h in (hA, hB):
                                ps, e_t = st[h]
                                for j in p2_js:
                                    smm(ps, h, j)
                                sexp(ps, e_t, p2_js)

                        return finish
                    for h in (hA, hB):
                        ps = sps_pool.tile([128, nch, 512], F32,
                                           tag="s", name="sps")
                        e_t = get_e(h, t)
                        for j in range(nch):
                            smm(ps, h, j)
                        sexp(ps, e_t, None)
                        if filler:
                            filler()
                    return None

                def av_piece(h, tts, j, first, last, fold=False):
                    # one AV accumulation group: head h, k-tiles tts,
                    # chunk j.  With fold=True (the h2/h3 tail), the
                    # previous acc value is folded in via an identity
                    # matmul so the evacuation is a plain copy that can
                    # run on ACT (idle after the last exp) parallel to DVE.
                    off, w = chunks[j]
                    pool = sps_pool if fold and j < 2 else ops
                    avp = pool.tile([DH + 1, 512], F32,
                                    tag="s" if pool is sps_pool else "o",
                                    name="av")
                    if fold:
                        nc.tensor.matmul(
                            avp[:, 0:w], ident_sb[:],
                            acc_sb[:, h, off:off + w],
                            start=True, stop=False,
                        )
                    for i, t0 in enumerate(tts):
                        st = (i == 0) and not fold
                        sp = (i == len(tts) - 1)
                        mt = mrows(t0)
                        nc.tensor.matmul(
                            avp[:, 0:w],
                            v_sb[0:mt, t0, h, 0:DH + 1],
                            e_tiles[(h, t0)][0:mt, off:off + w],
                            start=st, stop=sp,
                        )
                    if fold:
                        # even head of the pair finishes an exp earlier ->
                        # its evacs go to DVE (free immediately); the odd
                        # head's go to ACT (free right after its last exp)
                        if h % 2 == 1:
                            nc.scalar.activation(
                                acc_sb[:, h, off:off + w], avp[:, 0:w],
                                AF.Identity, bias=0.0)
                        else:
                            nc.vector.tensor_copy(
                                acc_sb[:, h, off:off + w], avp[:, 0:w])
                    elif first:
                        nc.vector.tensor_copy(
                            acc_sb[:, h, off:off + w], avp[:, 0:w])
                    else:
                        nc.vector.tensor_add(
                            acc_sb[:, h, off:off + w],
                            acc_sb[:, h, off:off + w], avp[:, 0:w])
                    if last and not fold:
                        nc.sync.dma_start(
                            out_d.ap()[:, h, off:off + w].bitcast(F32R),
                            acc_sb[:, h, off:off + w])
                    elif last and fold and j == nch - 1:
                        # one whole-head DMA: saves serialized HWDGE slots
                        # on the critical tail
                        nc.sync.dma_start(
                            out_d.ap()[:, h, :].bitcast(F32R),
                            acc_sb[:, h, :])

                # ---- phase 1: h0/h1 scores+exps.  Tile 0 runs j-split:
                # its pass-1-chunk exps seed ACT right after the phase-A
                # evacs, the q-d0-j1 unit runs under them, then the j1
                # continuation.  All v and remaining qk projections
                # drip-feed as filler units under the ACT-paced stream.
                units = deque()
                for off, w in chunks[1:2]:
                    for o2 in range(off, off + w, 256):
                        units.append(("qk", (1, kt_sb, 2, 0, o2,
                                             min(256, off + w - o2))))
                for wi, o_sb, bcol in ((0, qt_sb, 0), (1, kt_sb, 2)):
                    for off, w in chunks:
                        for o2 in range(off, off + w, 256):
                            units.append(
                                ("qk", (wi, o_sb, bcol, 1, o2,
                                        min(256, off + w - o2))))
                for t in proc_t:
                    units.append(("v", t))

                def run_unit(kind, u):
                    if kind == "qk":
                        qk_unit(u)
                    else:
                        v_unit(u)

                def filler():
                    if units:
                        run_unit(*units.popleft())

                fin = scores_pair(0, 1, 0, jsplit=True)
                for off, w in chunks[1:2]:
                    for o2 in range(off, off + w, 256):
                        qk_unit((0, qt_sb, 0, 0, o2,
                                 min(256, off + w - o2)))
                if fin:
                    fin()
                for t in range(1, nkt):
                    scores_pair(0, 1, t, filler=filler)

                # ---- phase 2: h2/h3 scores in proc_t order with AV pieces
                # spread across the slots.  h0/h1 run one accumulation
                # round each (everything is ready at phase start); h2/h3
                # rounds are release-gated and only the small folded last
                # round trails the final exp.
                q01 = deque()
                for h in (0, 1):
                    for j in range(nch):
                        q01.append((h, list(range(nkt)), j, True, True,
                                    False))
                q23 = deque()
                nr23 = len(rounds23)

                def rel23(plo, phi):
                    for gi, (a, b) in enumerate(rounds23):
                        if plo < b <= phi:
                            fold = gi == nr23 - 1 and nr23 > 1
                            for j in range(nch):
                                for h in (2, 3):
                                    q23.append((h, proc_t[a:b], j, gi == 0,
                                                gi == nr23 - 1, fold))

                for p, t in enumerate(proc_t):
                    scores_pair(2, 3, t)
                    rel23(p, p + 1)
                    n = 1 if p < 3 else 2
                    if units:
                        run_unit(*units.popleft())
                        n -= 1
                    while n and (q23 or q01):
                        av_piece(*(q23.popleft() if q23 else q01.popleft()))
                        n -= 1
                while units:
                    run_unit(*units.popleft())
                while q01:
                    av_piece(*q01.popleft())
                rel23(nkt, 10 * nkt)
                while q23:
                    av_piece(*q23.popleft())

    nc.compile()
    return nc


def _get_nc(tp, nkt, ml, with_bv=False):
    key = (tp, nkt, ml, with_bv)
    if key not in _CACHE:
        _CACHE[key] = _build(tp, nkt, ml, with_bv)
    return _CACHE[key]


def _swizzle_w(w, cs):
    """[C, CSL] slice -> [128, 2, NCT, 128] (p, d, i, c) bf16."""
    a = np.ascontiguousarray(w[:, cs:cs + CSL]).reshape(NCT, 128, 2, 128)
    return np.ascontiguousarray(a.transpose(1, 2, 0, 3)).astype(NPFP16)


def kernel(x, Wq, bq, Wk, bk, Wv, bv, mask):
    x = np.asarray(x, dtype=np.float32)
    Wq = np.asarray(Wq, dtype=np.float32)
    bq = np.asarray(bq, dtype=np.float32)
    Wk = np.asarray(Wk, dtype=np.float32)
    bk = np.asarray(bk, dtype=np.float32)
    Wv = np.asarray(Wv, dtype=np.float32)
    bv = np.asarray(bv, dtype=np.float32)
    mask = np.asarray(mask)

    idxs = [np.nonzero(mask[b] != 0)[0] for b in range(B)]
    tvs = [len(ix) for ix in idxs]
    tp, nkt, ml = _pick_dims(max(max(tvs), 1))
    with_bv = bool(np.any(bv))
    nc = _get_nc(tp, nkt, ml, with_bv)

    onesv = np.ones((128, nkt * HPC), NPFP16)

    # per-batch tensors
    xts, ebs = [], []
    for b in range(B):
        xt = np.zeros((C, tp), np.float32)
        if tvs[b]:
            xt[:, :tvs[b]] = x[b][idxs[b]].T
        xts.append(np.ascontiguousarray(
            xt.reshape(NCT, 128, tp).transpose(1, 0, 2)).astype(NPFP16))
        eb = np.full(nkt * 128, -1e30, np.float32)
        eb[:tvs[b]] = EB
        ebs.append(np.ascontiguousarray(eb.reshape(nkt, 128).T))

    in_maps = []
    for core in range(N_CORES):
        b, hg = core // HPC, core % HPC
        cs = hg * CSL
        bias128 = np.concatenate([
            np.stack([bq[cs:cs + 128], bq[cs + 128:cs + 256],
                      bk[cs:cs + 128], bk[cs + 128:cs + 256]], axis=1),
            ebs[b],
        ], axis=1)
        m = {
            "xt": xts[b],
            "wq": _swizzle_w(Wq, cs),
            "wk": _swizzle_w(Wk, cs),
            "wv": np.ascontiguousarray(
                Wv[:, cs:cs + CSL].reshape(NCT, 128, CSL)
                .transpose(1, 0, 2)).astype(NPFP16),
            "bias128": np.ascontiguousarray(bias128),
            "onesv": onesv,
            "ident": np.eye(DH + 1, dtype=np.float32),
        }
        if with_bv:
            m["misc1"] = np.concatenate(
                [bv[cs:cs + CSL], np.ones(128, np.float32)]
            ).reshape(1, -1).astype(NPFP16)
        in_maps.append(m)

    try:
        res = bass_utils.run_bass_kernel_spmd(
            nc, in_maps, core_ids=list(range(N_CORES)), trace=False)
    except Exception:
        # transient axon-worker/NRT failures recover on retry
        res = bass_utils.run_bass_kernel_spmd(
            nc, in_maps, core_ids=list(range(N_CORES)), trace=False)

    y = np.zeros((B, T, C), np.float32)
    for core in range(N_CORES):
        b, hg = core // HPC, core % HPC
        out = res.results[core]["out"]          # [DH+1, HPC, tp]
        ix, tv = idxs[b], tvs[b]
        if not tv:
            continue
        for h in range(HPC):
            numer = out[:DH, h, :tv]
            denom = out[DH, h, :tv]
            col = hg * CSL + h * DH
            y[b, ix, col:col + DH] = (numer / denom).T
    return y


# revision 71
# speedup vs baseline: 1.0012x; 1.0012x over previous
"""Bass/Trainium2 kernel for masked (padding) multi-head self-attention.

Problem: B=2, T=2048, C=1024, H=16 heads of DH=64.
  q/k/v = x @ W* + b*  ->  att = softmax(mask(q k^T / 8))  ->  y = att @ v

Sharding over 8 NeuronCores: core = (batch b, head-group hg) with
b = core // 4, hg = core % 4; each core computes 4 heads for one batch
element (its [T, 256] slice of q/k/v from the Wq/Wk/Wv column slice).

Pipeline (per core):
  - Host gathers valid tokens (mask!=0), pads to TP = roundup(max_tv, 16)
    (1040 for the seed-0 mask vs 2048 raw; ragged last k-tile of
    ML = TP-1024 rows), transposes x, converts x^T/Wq/Wk/Wv to fp16 and
    pre-swizzles them into the exact SBUF layouts (full-rate DMA runs).
  - All matmul operands are fp16 (1 PE cycle/row at any free size, vs
    f32r needing >=256; ~0.05% quantization so softmax logit noise stays
    ~1e-3 -- fp8 DoubleRow was tried and FAILS the 2e-2 gate: its ~3%
    logit noise gives 5e-2 errors on near-tied attention rows, and
    e^(s-2) overflows e4m3 at the seed-0 max score).  PSUM stays fp32.
  - Phase A (under the x^T DMA, c-tile-major): q/k d0 projections for
    the PSUM-bank-aligned q-chunks [512, 512, TP-1024] minus the middle
    chunk; kT-j0 evacuates on ACT parallel to qT on DVE, so tile 0's
    scores run j-split [j0, j2 | j1] and the first exp fires ~11us.
  - Phase 1: h0/h1 scores s^T = k^T q per k-tile; ONE exp per tile reads
    the flat [128, TP] PSUM span into an fp16 e-tile (bias -2 folded in,
    cancels in the softmax ratio).  Remaining projections (q/k d1, the
    d0 middle chunk, v) drip-feed as ~0.85us filler units under the
    ACT-paced stream.
  - Phase 2: h2/h3 scores (ragged k-tile first) interleaved with AV
    pieces: per (head, chunk) accumulation groups over k-tiles, with an
    extra ones column (M=65) making output row 64 the softmax
    denominator.  h0/h1 run one 9-matmul round; h2/h3 three
    release-gated rounds whose small last round folds the previous acc
    via an identity matmul so its evacuation is a copy split across
    ACT/DVE, followed by one whole-head out-DMA.  Host divides
    numerator/denominator and scatters.

Cost-model timeline ~68.8us (vs 77.4us baseline): PE-bound, busy ~54us
(qkT 13.8 + v 7.7 + scores 15.6 + AV 15.6 + overheads), ACT exp stream
dense 12.7->54us, ~3.5us evac+DMA tail.  HW-verified rel err 8.9e-4.
"""

import sys
from collections import deque

sys.path.insert(0, "/opt/trn_rl_repo")

import ml_dtypes
import numpy as np

import concourse.bacc as bacc
import concourse.mybir as mybir
import concourse.tile as tile
from concourse import bass_utils

F32 = mybir.dt.float32
F32R = mybir.dt.float32r
FP16 = mybir.dt.float16
AF = mybir.ActivationFunctionType
NPFP16 = np.float16

B, T, C, H = 2, 2048, 1024, 16
DH = C // H            # 64
HPC = 4                # heads per core
CSL = HPC * DH         # 256, per-core column slice of C
N_CORES = 8
NCT = C // 128         # 8 contraction tiles over C
VW = 80                # padded v row width (DoubleRow needs stride%16==0)
EB = -2.0              # exp bias shift, cancels in softmax ratio

_CACHE: dict = {}


def _pick_dims(max_valid: int):
    """TP (multiple of 2 for fp16 DMA runs, >=128), k-tiles, last rows."""
    tp = max(-(-max_valid // 2) * 2, 128)
    nkt = -(-tp // 128)
    ml = tp - 128 * (nkt - 1)
    return tp, nkt, ml


def _chunks(tp: int):
    """PSUM-bank-aligned q-chunks: [512, 512, ...] + ragged tail."""
    return [(o, min(512, tp - o)) for o in range(0, tp, 512)]


def _build(tp: int, nkt: int, ml: int, with_bv: bool = False):
    nc = bacc.Bacc("TRN2", target_bir_lowering=False, debug=False,
                   num_devices=N_CORES)

    chunks = _chunks(tp)
    nch = len(chunks)

    # phase-2 processing order: the ragged k-tile first, so the last
    # h2/h3 AV round (the tail after the final exp) covers full tiles
    proc_t = [nkt - 1] + list(range(nkt - 1)) if nkt > 1 else [0]
    # h0/h1 AV: one accumulation round over all k-tiles (everything is
    # ready when phase 2 starts).  h2/h3: three release-gated rounds,
    # the last one small, in proc_t position space [a, b).
    if nkt >= 5:
        rounds23 = [(0, 3), (3, nkt - 2), (nkt - 2, nkt)]
    else:
        rounds23 = [(0, nkt)]

    # host-pre-swizzled inputs (partition-first layouts)
    xt_d = nc.dram_tensor("xt", [128, NCT, tp], FP16, kind="ExternalInput")
    wq_d = nc.dram_tensor("wq", [128, 2, NCT, 128], FP16, kind="ExternalInput")
    wk_d = nc.dram_tensor("wk", [128, 2, NCT, 128], FP16, kind="ExternalInput")
    wv_d = nc.dram_tensor("wv", [128, NCT, CSL], FP16, kind="ExternalInput")
    # bias128: col 0..3 = bqk (bq d0, bq d1, bk d0, bk d1), col 4.. = ebias
    bias128_d = nc.dram_tensor("bias128", [128, 4 + nkt], F32,
                               kind="ExternalInput")
    onesv_d = nc.dram_tensor("onesv", [128, nkt * HPC], FP16,
                             kind="ExternalInput")
    ident_d = nc.dram_tensor("ident", [DH + 1, DH + 1], F32,
                             kind="ExternalInput")
    if with_bv:
        misc1_d = nc.dram_tensor("misc1", [1, CSL + 128], FP16,
                                 kind="ExternalInput")
    out_d = nc.dram_tensor("out", [DH + 1, HPC, tp], F32,
                           kind="ExternalOutput")

    def mrows(t):
        return ml if t == nkt - 1 else 128

    with tile.TileContext(nc) as tc:
        with tc.tile_pool(name="const", bufs=1) as cp:
            xt_sb = cp.tile([128, NCT, tp], FP16, tag="xt")
            wq_sb = cp.tile([128, 2, NCT, 128], FP16, tag="wq")
            wk_sb = cp.tile([128, 2, NCT, 128], FP16, tag="wk")
            wv_sb = cp.tile([128, NCT, CSL], FP16, tag="wv")
            bias128_sb = cp.tile([128, 4 + nkt], F32, tag="bias128")
            qt_sb = cp.tile([128, 2, tp], FP16, tag="qt")
            kt_sb = cp.tile([128, 2, tp], FP16, tag="kt")
            v_sb = cp.tile([128, nkt, HPC, VW], FP16, tag="v")
            acc_sb = cp.tile([DH + 1, HPC, tp], F32R, tag="acc")
            ident_sb = cp.tile([DH + 1, DH + 1], F32R,
                               tag="ident")
            bqk_sb = bias128_sb[:, 0:4]
            ebias_sb = bias128_sb[:, 4:4 + nkt]
            if with_bv:
                misc1_sb = cp.tile([1, CSL + 128], FP16, tag="misc1")
                bv_sb = misc1_sb[:, 0:CSL]
                ones_sb = misc1_sb[:, CSL:CSL + 128]

            scratch = cp.tile([1, 8], F32, tag="scratch")

            # critical-path DMA order: wq d0 (full-rate thanks to the host
            # swizzle), bias, then x^T c-tiles with wk d0 slotted after c0;
            # everything else after x^T completes.
            nc.sync.dma_start(wq_sb[:, 0], wq_d.ap()[:, 0])
            nc.sync.dma_start(bias128_sb[:], bias128_d.ap()[:])
            nc.sync.dma_start(xt_sb[:, 0], xt_d.ap()[:, 0])
            nc.sync.dma_start(wk_sb[:, 0], wk_d.ap()[:, 0])
            for i in range(1, NCT):
                nc.sync.dma_start(xt_sb[:, i], xt_d.ap()[:, i])
            nc.sync.dma_start(wq_sb[:, 1], wq_d.ap()[:, 1])
            nc.sync.dma_start(wk_sb[:, 1], wk_d.ap()[:, 1])
            nc.sync.dma_start(wv_sb[:], wv_d.ap()[:])
            nc.sync.dma_start(
                v_sb[:, :, :, DH],
                onesv_d.ap().rearrange("p (t h) -> p t h", h=HPC))
            nc.sync.dma_start(
                ident_sb[:], ident_d.ap()[:].bitcast(F32R))
            if with_bv:
                nc.sync.dma_start(misc1_sb[:], misc1_d.ap()[:])

            # tiny PE-ramp dummy source memset first (dummies wait on
            # it), then the ACT exp table warm, all during the DMA window
            wsc = cp.tile([128, 16], FP16, tag="wsc")
            nc.gpsimd.memset(wsc[:], 0.0)
            nc.gpsimd.memset(scratch[:], 0.0)
            nc.scalar.activation(scratch[:], scratch[:], AF.Exp)

            def evac_qk(o_sb, d, off, w, ps, bcol, on_act=False):
                if on_act:
                    nc.scalar.activation(
                        o_sb[:, d, off:off + w], ps[:, 0:w], AF.Identity,
                        bias=bqk_sb[:, bcol + d:bcol + d + 1])
                else:
                    nc.vector.tensor_scalar_add(
                        o_sb[:, d, off:off + w], ps[:, 0:w],
                        bqk_sb[:, bcol + d:bcol + d + 1])

            # phase A, two passes.  Pass 1 (c-tile-major, pipelining with
            # the x^T DMA): the j0 + last-chunk d0 groups whose evacs gate
            # the first exps, plus q-d1-j0 as filler.  Pass 2 (PE backlog
            # right after): the q/k d0 j1 groups, q first — tile 0's exps
            # run j-split [j0, j2, then j1] so ACT starts ~3.5us earlier.
            # kT-j0 evacuates on ACT, everything else on DVE.
            p1_js = [0] + ([nch - 1] if nch > 2 else [])
            p2_js = [j for j in range(1, nch) if j not in p1_js]
            pa_p1 = [(wi, 0, j) for j in p1_js for wi in (0, 1)]
            pa_p2 = []
            with tc.tile_pool(name="pa", bufs=len(pa_p1) + len(pa_p2),
                              space="PSUM") as pa:
                # bridge the PE p-state ramp during the DMA head
                for _ in range(40):
                    wps = pa.tile([16, 16], F32, tag="a", name="wps")
                    nc.tensor.matmul(wps[:], wsc[:], wsc[:],
                                     start=True, stop=True)
                pga = {}
                for spec in pa_p1 + pa_p2:
                    pga[spec] = pa.tile([128, 512], F32, tag="a",
                                        name="pqk0")

                def pa_mm(spec, ct):
                    wi, d, j = spec
                    w_sb = wq_sb if wi == 0 else wk_sb
                    off, w = chunks[j]
                    nc.tensor.matmul(
                        pga[spec][:, 0:w],
                        w_sb[:, d, ct, :],
                        xt_sb[:, ct, off:off + w],
                        start=(ct == 0), stop=(ct == NCT - 1),
                    )

                for ct in range(NCT):
                    for spec in pa_p1:
                        pa_mm(spec, ct)
                evac_qk(kt_sb, 0, chunks[0][0], chunks[0][1],
                        pga[(1, 0, 0)], 2, on_act=True)
                for j in p1_js:
                    off, w = chunks[j]
                    evac_qk(qt_sb, 0, off, w, pga[(0, 0, j)], 0)
                for j in p1_js[1:]:
                    off, w = chunks[j]
                    evac_qk(kt_sb, 0, off, w, pga[(1, 0, j)], 2)


            ebufs = 4 * nkt + 2

            with (
                tc.tile_pool(name="ops", bufs=2, space="PSUM") as ops,
                tc.tile_pool(name="sps", bufs=2, space="PSUM") as sps_pool,
                tc.tile_pool(name="epool", bufs=ebufs) as ep,
            ):
                e_tiles: dict = {}

                def get_e(h, t):
                    if (h, t) not in e_tiles:
                        e_tiles[(h, t)] = ep.tile([128, tp], FP16,
                                                  tag="e", name="e")
                    return e_tiles[(h, t)]

                def qk_unit(u):
                    # 256-wide column sub-chunks (full contraction) so each
                    # filler unit is only ~0.85us of PE work
                    wi, o_sb, bcol, d, off, w = u
                    w_sb = wq_sb if wi == 0 else wk_sb
                    ps = ops.tile([128, 256], F32, tag="o", name="pqk1")
                    for ct in range(NCT):
                        nc.tensor.matmul(
                            ps[:, 0:w],
                            w_sb[:, d, ct, :],
                            xt_sb[:, ct, off:off + w],
                            start=(ct == 0), stop=(ct == NCT - 1),
                        )
                    evac_qk(o_sb, d, off, w, ps, bcol)

                def v_unit(t):
                    mt = mrows(t)
                    ps = ops.tile([128, CSL], F32, tag="o", name="pv")
                    for ct in range(NCT):
                        nc.tensor.matmul(
                            ps[0:mt, :],
                            xt_sb[:, ct, 128 * t:128 * t + mt],
                            wv_sb[:, ct, :],
                            start=(ct == 0),
                            stop=(not with_bv and ct == NCT - 1),
                        )
                    if with_bv:
                        nc.tensor.matmul(ps[0:mt, :], ones_sb[:, 0:mt],
                                         bv_sb[:], start=False, stop=True)
                    nc.vector.tensor_copy(
                        v_sb[0:mt, t, :, 0:DH],
                        ps[0:mt, :].rearrange("p (h d) -> p h d", h=HPC),
                    )

                def scores_pair(hA, hB, t, filler=None, jsplit=False):
                    # hA/hB share a qT/kT d-tile at partition offsets 0/64.
                    # Per-head emission: the head's chunk matmuls then its
                    # exp, so ACT is fed after only 3 matmuls; the filler
                    # hook runs after each head.  jsplit (tile 0): emit the
                    # pass-1 chunks' matmuls+exps for BOTH heads first, the
                    # pass-2 chunk (whose qT evacuates later) behind them.
                    mt = mrows(t)
                    pd = hA // 2

                    def smm(ps, h, j):
                        off, w = chunks[j]
                        po = (h % 2) * 64
                        nc.tensor.matmul(
                            ps[0:mt, j, 0:w],
                            kt_sb[po:po + 64, pd, 128 * t:128 * t + mt],
                            qt_sb[po:po + 64, pd, off:off + w],
                            start=True, stop=True,
                        )

                    def sexp(ps, e_t, js):
                        flat = ps.rearrange("p a b -> p (a b)")
                        if js is None:
                            nc.scalar.activation(
                                e_t[:, 0:tp], flat[:, 0:tp], AF.Exp,
                                bias=ebias_sb[:, t:t + 1], scale=0.125,
                            )
                            return
                        for j in js:
                            off, w = chunks[j]
                            nc.scalar.activation(
                                e_t[:, off:off + w],
                                flat[:, off:off + w], AF.Exp,
                                bias=ebias_sb[:, t:t + 1], scale=0.125,
                            )

                    if jsplit and nch > 1:
                        st = {}
                        for h in (hA, hB):
                            st[h] = (sps_pool.tile([128, nch, 512], F32,
                                                   tag="s", name="sps"),
                                     get_e(h, t))
                        for h in (hA, hB):
                            ps, e_t = st[h]
                            for j in p1_js:
                                smm(ps, h, j)
                            sexp(ps, e_t, p1_js)

                        def finish():
                            for h in (hA, hB):
                                ps, e_t = st[h]
                                for j in p2_js:
                                    smm(ps, h, j)
                                sexp(ps, e_t, p2_js)

                        return finish
                    for h in (hA, hB):
                        ps = sps_pool.tile([128, nch, 512], F32,
                                           tag="s", name="sps")
                        e_t = get_e(h, t)
                        for j in range(nch):
                            smm(ps, h, j)
                        sexp(ps, e_t, None)
                        if filler:
                            filler()
                    return None

                def av_piece(h, tts, j, first, last, fold=False):
                    # one AV accumulation group: head h, k-tiles tts,
                    # chunk j.  With fold=True (the h2/h3 tail), the
                    # previous acc value is folded in via an identity
                    # matmul so the evacuation is a plain copy that can
                    # run on ACT (idle after the last exp) parallel to DVE.
                    off, w = chunks[j]
                    pool = sps_pool if fold and j < 2 else ops
                    avp = pool.tile([DH + 1, 512], F32,
                                    tag="s" if pool is sps_pool else "o",
                                    name="av")
                    if fold:
                        nc.tensor.matmul(
                            avp[:, 0:w], ident_sb[:],
                            acc_sb[:, h, off:off + w],
                            start=True, stop=False,
                        )
                    for i, t0 in enumerate(tts):
                        st = (i == 0) and not fold
                        sp = (i == len(tts) - 1)
                        mt = mrows(t0)
                        nc.tensor.matmul(
                            avp[:, 0:w],
                            v_sb[0:mt, t0, h, 0:DH + 1],
                            e_tiles[(h, t0)][0:mt, off:off + w],
                            start=st, stop=sp,
                        )
                    if fold:
                        # even head of the pair finishes an exp earlier ->
                        # its evacs go to DVE (free immediately); the odd
                        # head's go to ACT (free right after its last exp)
                        if h % 2 == 1:
                            nc.scalar.activation(
                                acc_sb[:, h, off:off + w], avp[:, 0:w],
                                AF.Identity, bias=0.0)
                        else:
                            nc.vector.tensor_copy(
                                acc_sb[:, h, off:off + w], avp[:, 0:w])
                    elif first:
                        nc.vector.tensor_copy(
                            acc_sb[:, h, off:off + w], avp[:, 0:w])
                    else:
                        nc.vector.tensor_add(
                            acc_sb[:, h, off:off + w],
                            acc_sb[:, h, off:off + w], avp[:, 0:w])
                    if last and not fold:
                        nc.sync.dma_start(
                            out_d.ap()[:, h, off:off + w].bitcast(F32R),
                            acc_sb[:, h, off:off + w])
                    elif last and fold and j == nch - 1:
                        # one whole-head DMA: saves serialized HWDGE slots
                        # on the critical tail
                        nc.sync.dma_start(
                            out_d.ap()[:, h, :].bitcast(F32R),
                            acc_sb[:, h, :])

                # ---- phase 1: h0/h1 scores+exps.  Tile 0 runs j-split:
                # its pass-1-chunk exps seed ACT right after the phase-A
                # evacs, the q-d0-j1 unit runs under them, then the j1
                # continuation.  All v and remaining qk projections
                # drip-feed as filler units under the ACT-paced stream.
                units = deque()
                for off, w in chunks[1:2]:
                    for o2 in range(off, off + w, 256):
                        units.append(("qk", (1, kt_sb, 2, 0, o2,
                                             min(256, off + w - o2))))
                for wi, o_sb, bcol in ((0, qt_sb, 0), (1, kt_sb, 2)):
                    for off, w in chunks:
                        for o2 in range(off, off + w, 256):
                            units.append(
                                ("qk", (wi, o_sb, bcol, 1, o2,
                                        min(256, off + w - o2))))
                for t in proc_t:
                    units.append(("v", t))

                def run_unit(kind, u):
                    if kind == "qk":
                        qk_unit(u)
                    else:
                        v_unit(u)

                def filler():
                    if units:
                        run_unit(*units.popleft())

                fin = scores_pair(0, 1, 0, jsplit=True)
                for off, w in chunks[1:2]:
                    for o2 in range(off, off + w, 256):
                        qk_unit((0, qt_sb, 0, 0, o2,
                                 min(256, off + w - o2)))
                if fin:
                    fin()
                for t in range(1, nkt):
                    scores_pair(0, 1, t, filler=filler)

                # ---- phase 2: h2/h3 scores in proc_t order with AV pieces
                # spread across the slots.  h0/h1 run one accumulation
                # round each (everything is ready at phase start); h2/h3
                # rounds are release-gated and only the small folded last
                # round trails the final exp.
                q01 = deque()
                for h in (0, 1):
                    for j in range(nch):
                        q01.append((h, list(range(nkt)), j, True, True,
                                    False))
                q23 = deque()
                nr23 = len(rounds23)

                def rel23(plo, phi):
                    for gi, (a, b) in enumerate(rounds23):
                        if plo < b <= phi:
                            fold = gi == nr23 - 1 and nr23 > 1
                            for j in range(nch):
                                for h in (2, 3):
                                    q23.append((h, proc_t[a:b], j, gi == 0,
                                                gi == nr23 - 1, fold))

                for p, t in enumerate(proc_t):
                    scores_pair(2, 3, t)
                    rel23(p, p + 1)
                    n = 1 if p < 3 else 2
                    if units:
                        run_unit(*units.popleft())
                        n -= 1
                    while n and (q23 or q01):
                        av_piece(*(q23.popleft() if q23 else q01.popleft()))
                        n -= 1
                while units:
                    run_unit(*units.popleft())
                while q01:
                    av_piece(*q01.popleft())
                rel23(nkt, 10 * nkt)
                while q23:
                    av_piece(*q23.popleft())

    nc.compile()
    return nc


def _get_nc(tp, nkt, ml, with_bv=False):
    key = (tp, nkt, ml, with_bv)
    if key not in _CACHE:
        _CACHE[key] = _build(tp, nkt, ml, with_bv)
    return _CACHE[key]


def _swizzle_w(w, cs):
    """[C, CSL] slice -> [128, 2, NCT, 128] (p, d, i, c) bf16."""
    a = np.ascontiguousarray(w[:, cs:cs + CSL]).reshape(NCT, 128, 2, 128)
    return np.ascontiguousarray(a.transpose(1, 2, 0, 3)).astype(NPFP16)


def kernel(x, Wq, bq, Wk, bk, Wv, bv, mask):
    x = np.asarray(x, dtype=np.float32)
    Wq = np.asarray(Wq, dtype=np.float32)
    bq = np.asarray(bq, dtype=np.float32)
    Wk = np.asarray(Wk, dtype=np.float32)
    bk = np.asarray(bk, dtype=np.float32)
    Wv = np.asarray(Wv, dtype=np.float32)
    bv = np.asarray(bv, dtype=np.float32)
    mask = np.asarray(mask)

    idxs = [np.nonzero(mask[b] != 0)[0] for b in range(B)]
    tvs = [len(ix) for ix in idxs]
    tp, nkt, ml = _pick_dims(max(max(tvs), 1))
    with_bv = bool(np.any(bv))
    nc = _get_nc(tp, nkt, ml, with_bv)

    onesv = np.ones((128, nkt * HPC), NPFP16)

    # per-batch tensors
    xts, ebs = [], []
    for b in range(B):
        xt = np.zeros((C, tp), np.float32)
        if tvs[b]:
            xt[:, :tvs[b]] = x[b][idxs[b]].T
        xts.append(np.ascontiguousarray(
            xt.reshape(NCT, 128, tp).transpose(1, 0, 2)).astype(NPFP16))
        eb = np.full(nkt * 128, -1e30, np.float32)
        eb[:tvs[b]] = EB
        ebs.append(np.ascontiguousarray(eb.reshape(nkt, 128).T))

    in_maps = []
    for core in range(N_CORES):
        b, hg = core // HPC, core % HPC
        cs = hg * CSL
        bias128 = np.concatenate([
            np.stack([bq[cs:cs + 128], bq[cs + 128:cs + 256],
                      bk[cs:cs + 128], bk[cs + 128:cs + 256]], axis=1),
            ebs[b],
        ], axis=1)
        m = {
            "xt": xts[b],
            "wq": _swizzle_w(Wq, cs),
            "wk": _swizzle_w(Wk, cs),
            "wv": np.ascontiguousarray(
                Wv[:, cs:cs + CSL].reshape(NCT, 128, CSL)
                .transpose(1, 0, 2)).astype(NPFP16),
            "bias128": np.ascontiguousarray(bias128),
            "onesv": onesv,
            "ident": np.eye(DH + 1, dtype=np.float32),
        }
        if with_bv:
            m["misc1"] = np.concatenate(
                [bv[cs:cs + CSL], np.ones(128, np.float32)]
            ).reshape(1, -1).astype(NPFP16)
        in_maps.append(m)

    try:
        res = bass_utils.run_bass_kernel_spmd(
            nc, in_maps, core_ids=list(range(N_CORES)), trace=False)
    except Exception:
        # transient axon-worker/NRT failures recover on retry
        res = bass_utils.run_bass_kernel_spmd(
            nc, in_maps, core_ids=list(range(N_CORES)), trace=False)

    y = np.zeros((B, T, C), np.float32)
    for core in range(N_CORES):
        b, hg = core // HPC, core % HPC
        out = res.results[core]["out"]          # [DH+1, HPC, tp]
        ix, tv = idxs[b], tvs[b]
        if not tv:
            continue
        for h in range(HPC):
            numer = out[:DH, h, :tv]
            denom = out[DH, h, :tv]
            col = hg * CSL + h * DH
            y[b, ix, col:col + DH] = (numer / denom).T
    return y


# revision 72
# speedup vs baseline: 1.0096x; 1.0084x over previous
"""Bass/Trainium2 kernel for masked (padding) multi-head self-attention.

Problem: B=2, T=2048, C=1024, H=16 heads of DH=64.
  q/k/v = x @ W* + b*  ->  att = softmax(mask(q k^T / 8))  ->  y = att @ v

Sharding over 8 NeuronCores: core = (batch b, head-group hg) with
b = core // 4, hg = core % 4; each core computes 4 heads for one batch
element (its [T, 256] slice of q/k/v from the Wq/Wk/Wv column slice).

Pipeline (per core):
  - Host gathers valid tokens (mask!=0), pads to TP = roundup(max_tv, 16)
    (1040 for the seed-0 mask vs 2048 raw; ragged last k-tile of
    ML = TP-1024 rows), transposes x, converts x^T/Wq/Wk/Wv to fp16 and
    pre-swizzles them into the exact SBUF layouts (full-rate DMA runs).
  - All matmul operands are fp16 (1 PE cycle/row at any free size, vs
    f32r needing >=256; ~0.05% quantization so softmax logit noise stays
    ~1e-3 -- fp8 DoubleRow was tried and FAILS the 2e-2 gate: its ~3%
    logit noise gives 5e-2 errors on near-tied attention rows, and
    e^(s-2) overflows e4m3 at the seed-0 max score).  PSUM stays fp32.
  - Phase A (under the x^T DMA, c-tile-major): q/k d0 projections for
    the PSUM-bank-aligned q-chunks [512, 512, TP-1024] minus the middle
    chunk; kT-j0 evacuates on ACT parallel to qT on DVE, so tile 0's
    scores run j-split [j0, j2 | j1] and the first exp fires ~11us.
  - Phase 1: h0/h1 scores s^T = k^T q per k-tile; ONE exp per tile reads
    the flat [128, TP] PSUM span into an fp16 e-tile (bias -2 folded in,
    cancels in the softmax ratio).  Remaining projections (q/k d1, the
    d0 middle chunk, v) drip-feed as ~0.85us filler units under the
    ACT-paced stream.
  - Phase 2: h2/h3 scores (ragged k-tile first) interleaved with AV
    pieces: per (head, chunk) accumulation groups over k-tiles, with an
    extra ones column (M=65) making output row 64 the softmax
    denominator.  h0/h1 run one 9-matmul round; h2/h3 three
    release-gated rounds whose small last round folds the previous acc
    via an identity matmul so its evacuation is a copy split across
    ACT/DVE, followed by one whole-head out-DMA.  Host divides
    numerator/denominator and scatters.

Cost-model timeline ~68.8us (vs 77.4us baseline): PE-bound, busy ~54us
(qkT 13.8 + v 7.7 + scores 15.6 + AV 15.6 + overheads), ACT exp stream
dense 12.7->54us, ~3.5us evac+DMA tail.  HW-verified rel err 8.9e-4.
"""

import sys
from collections import deque

sys.path.insert(0, "/opt/trn_rl_repo")

import ml_dtypes
import numpy as np

import concourse.bacc as bacc
import concourse.mybir as mybir
import concourse.tile as tile
from concourse import bass_utils

F32 = mybir.dt.float32
F32R = mybir.dt.float32r
FP16 = mybir.dt.float16
AF = mybir.ActivationFunctionType
NPFP16 = np.float16

B, T, C, H = 2, 2048, 1024, 16
DH = C // H            # 64
HPC = 4                # heads per core
CSL = HPC * DH         # 256, per-core column slice of C
N_CORES = 8
NCT = C // 128         # 8 contraction tiles over C
VW = 80                # padded v row width (DoubleRow needs stride%16==0)
EB = -2.0              # exp bias shift, cancels in softmax ratio

_CACHE: dict = {}


def _pick_dims(max_valid: int):
    """TP (multiple of 2 for fp16 DMA runs, >=128), k-tiles, last rows."""
    tp = max(-(-max_valid // 2) * 2, 128)
    nkt = -(-tp // 128)
    ml = tp - 128 * (nkt - 1)
    return tp, nkt, ml


def _chunks(tp: int):
    """PSUM-bank-aligned q-chunks: [512, 512, ...] + ragged tail."""
    return [(o, min(512, tp - o)) for o in range(0, tp, 512)]


def _build(tp: int, nkt: int, ml: int, with_bv: bool = False):
    nc = bacc.Bacc("TRN2", target_bir_lowering=False, debug=False,
                   num_devices=N_CORES)

    chunks = _chunks(tp)
    nch = len(chunks)

    # phase-2 processing order: the ragged k-tile first, so the last
    # h2/h3 AV round (the tail after the final exp) covers full tiles
    proc_t = [nkt - 1] + list(range(nkt - 1)) if nkt > 1 else [0]
    # h0/h1 AV: one accumulation round over all k-tiles (everything is
    # ready when phase 2 starts).  h2/h3: three release-gated rounds,
    # the last one small, in proc_t position space [a, b).
    if nkt >= 5:
        rounds23 = [(0, 3), (3, nkt - 2), (nkt - 2, nkt)]
    else:
        rounds23 = [(0, nkt)]

    # host-pre-swizzled inputs (partition-first layouts)
    xt_d = nc.dram_tensor("xt", [128, NCT, tp], FP16, kind="ExternalInput")
    wq_d = nc.dram_tensor("wq", [128, 2, NCT, 128], FP16, kind="ExternalInput")
    wk_d = nc.dram_tensor("wk", [128, 2, NCT, 128], FP16, kind="ExternalInput")
    wv_d = nc.dram_tensor("wv", [128, NCT, CSL], FP16, kind="ExternalInput")
    # bias128: col 0..3 = bqk (bq d0, bq d1, bk d0, bk d1), col 4.. = ebias
    bias128_d = nc.dram_tensor("bias128", [128, 4 + nkt], F32,
                               kind="ExternalInput")
    onesv_d = nc.dram_tensor("onesv", [128, nkt * HPC], FP16,
                             kind="ExternalInput")
    ident_d = nc.dram_tensor("ident", [DH + 1, DH + 1], F32,
                             kind="ExternalInput")
    if with_bv:
        misc1_d = nc.dram_tensor("misc1", [1, CSL + 128], FP16,
                                 kind="ExternalInput")
    out_d = nc.dram_tensor("out", [DH + 1, HPC, tp], F32,
                           kind="ExternalOutput")

    def mrows(t):
        return ml if t == nkt - 1 else 128

    with tile.TileContext(nc) as tc:
        with tc.tile_pool(name="const", bufs=1) as cp:
            xt_sb = cp.tile([128, NCT, tp], FP16, tag="xt")
            wq_sb = cp.tile([128, 2, NCT, 128], FP16, tag="wq")
            wk_sb = cp.tile([128, 2, NCT, 128], FP16, tag="wk")
            wv_sb = cp.tile([128, NCT, CSL], FP16, tag="wv")
            bias128_sb = cp.tile([128, 4 + nkt], F32, tag="bias128")
            qt_sb = cp.tile([128, 2, tp], FP16, tag="qt")
            kt_sb = cp.tile([128, 2, tp], FP16, tag="kt")
            v_sb = cp.tile([128, nkt, HPC, VW], FP16, tag="v")
            acc_sb = cp.tile([DH + 1, HPC, tp], F32R, tag="acc")
            ident_sb = cp.tile([DH + 1, DH + 1], F32R,
                               tag="ident")
            bqk_sb = bias128_sb[:, 0:4]
            ebias_sb = bias128_sb[:, 4:4 + nkt]
            if with_bv:
                misc1_sb = cp.tile([1, CSL + 128], FP16, tag="misc1")
                bv_sb = misc1_sb[:, 0:CSL]
                ones_sb = misc1_sb[:, CSL:CSL + 128]

            scratch = cp.tile([1, 8], F32, tag="scratch")

            # critical-path DMA order: wq d0 (full-rate thanks to the host
            # swizzle), bias, then x^T c-tiles with wk d0 slotted after c0;
            # everything else after x^T completes.
            nc.sync.dma_start(wq_sb[:, 0], wq_d.ap()[:, 0])
            nc.sync.dma_start(xt_sb[:, 0], xt_d.ap()[:, 0])
            nc.sync.dma_start(wk_sb[:, 0], wk_d.ap()[:, 0])
            for i in range(1, NCT):
                nc.sync.dma_start(xt_sb[:, i], xt_d.ap()[:, i])
            nc.sync.dma_start(bias128_sb[:], bias128_d.ap()[:])
            nc.sync.dma_start(wq_sb[:, 1], wq_d.ap()[:, 1])
            nc.sync.dma_start(wk_sb[:, 1], wk_d.ap()[:, 1])
            nc.sync.dma_start(wv_sb[:], wv_d.ap()[:])
            nc.sync.dma_start(
                v_sb[:, :, :, DH],
                onesv_d.ap().rearrange("p (t h) -> p t h", h=HPC))
            nc.sync.dma_start(
                ident_sb[:], ident_d.ap()[:].bitcast(F32R))
            if with_bv:
                nc.sync.dma_start(misc1_sb[:], misc1_d.ap()[:])

            # tiny PE-ramp dummy source memset first (dummies wait on
            # it), then the ACT exp table warm, all during the DMA window
            wsc = cp.tile([128, 16], FP16, tag="wsc")
            nc.gpsimd.memset(wsc[:], 0.0)
            nc.gpsimd.memset(scratch[:], 0.0)
            nc.scalar.activation(scratch[:], scratch[:], AF.Exp)

            def evac_qk(o_sb, d, off, w, ps, bcol, on_act=False):
                if on_act:
                    nc.scalar.activation(
                        o_sb[:, d, off:off + w], ps[:, 0:w], AF.Identity,
                        bias=bqk_sb[:, bcol + d:bcol + d + 1])
                else:
                    nc.vector.tensor_scalar_add(
                        o_sb[:, d, off:off + w], ps[:, 0:w],
                        bqk_sb[:, bcol + d:bcol + d + 1])

            # phase A, two passes.  Pass 1 (c-tile-major, pipelining with
            # the x^T DMA): the j0 + last-chunk d0 groups whose evacs gate
            # the first exps, plus q-d1-j0 as filler.  Pass 2 (PE backlog
            # right after): the q/k d0 j1 groups, q first — tile 0's exps
            # run j-split [j0, j2, then j1] so ACT starts ~3.5us earlier.
            # kT-j0 evacuates on ACT, everything else on DVE.
            p1_js = [0] + ([nch - 1] if nch > 2 else [])
            p2_js = [j for j in range(1, nch) if j not in p1_js]
            pa_p1 = [(wi, 0, j) for j in p1_js for wi in (0, 1)]
            pa_p2 = []
            with tc.tile_pool(name="pa", bufs=len(pa_p1) + len(pa_p2),
                              space="PSUM") as pa:
                # bridge the PE p-state ramp during the DMA head
                for _ in range(40):
                    wps = pa.tile([16, 16], F32, tag="a", name="wps")
                    nc.tensor.matmul(wps[:], wsc[:], wsc[:],
                                     start=True, stop=True)
                pga = {}
                for spec in pa_p1 + pa_p2:
                    pga[spec] = pa.tile([128, 512], F32, tag="a",
                                        name="pqk0")

                def pa_mm(spec, ct):
                    wi, d, j = spec
                    w_sb = wq_sb if wi == 0 else wk_sb
                    off, w = chunks[j]
                    nc.tensor.matmul(
                        pga[spec][:, 0:w],
                        w_sb[:, d, ct, :],
                        xt_sb[:, ct, off:off + w],
                        start=(ct == 0), stop=(ct == NCT - 1),
                    )

                for ct in range(NCT):
                    for spec in pa_p1:
                        pa_mm(spec, ct)
                evac_qk(kt_sb, 0, chunks[0][0], chunks[0][1],
                        pga[(1, 0, 0)], 2, on_act=True)
                for j in p1_js:
                    off, w = chunks[j]
                    evac_qk(qt_sb, 0, off, w, pga[(0, 0, j)], 0)
                for j in p1_js[1:]:
                    off, w = chunks[j]
                    evac_qk(kt_sb, 0, off, w, pga[(1, 0, j)], 2)


            ebufs = 4 * nkt + 2

            with (
                tc.tile_pool(name="ops", bufs=2, space="PSUM") as ops,
                tc.tile_pool(name="sps", bufs=2, space="PSUM") as sps_pool,
                tc.tile_pool(name="epool", bufs=ebufs) as ep,
            ):
                e_tiles: dict = {}

                def get_e(h, t):
                    if (h, t) not in e_tiles:
                        e_tiles[(h, t)] = ep.tile([128, tp], FP16,
                                                  tag="e", name="e")
                    return e_tiles[(h, t)]

                def qk_unit(u):
                    # 256-wide column sub-chunks (full contraction) so each
                    # filler unit is only ~0.85us of PE work
                    wi, o_sb, bcol, d, off, w = u
                    w_sb = wq_sb if wi == 0 else wk_sb
                    ps = ops.tile([128, 256], F32, tag="o", name="pqk1")
                    for ct in range(NCT):
                        nc.tensor.matmul(
                            ps[:, 0:w],
                            w_sb[:, d, ct, :],
                            xt_sb[:, ct, off:off + w],
                            start=(ct == 0), stop=(ct == NCT - 1),
                        )
                    evac_qk(o_sb, d, off, w, ps, bcol)

                def v_unit(t):
                    mt = mrows(t)
                    ps = ops.tile([128, CSL], F32, tag="o", name="pv")
                    for ct in range(NCT):
                        nc.tensor.matmul(
                            ps[0:mt, :],
                            xt_sb[:, ct, 128 * t:128 * t + mt],
                            wv_sb[:, ct, :],
                            start=(ct == 0),
                            stop=(not with_bv and ct == NCT - 1),
                        )
                    if with_bv:
                        nc.tensor.matmul(ps[0:mt, :], ones_sb[:, 0:mt],
                                         bv_sb[:], start=False, stop=True)
                    nc.vector.tensor_copy(
                        v_sb[0:mt, t, :, 0:DH],
                        ps[0:mt, :].rearrange("p (h d) -> p h d", h=HPC),
                    )

                def scores_pair(hA, hB, t, filler=None, jsplit=False):
                    # hA/hB share a qT/kT d-tile at partition offsets 0/64.
                    # Per-head emission: the head's chunk matmuls then its
                    # exp, so ACT is fed after only 3 matmuls; the filler
                    # hook runs after each head.  jsplit (tile 0): emit the
                    # pass-1 chunks' matmuls+exps for BOTH heads first, the
                    # pass-2 chunk (whose qT evacuates later) behind them.
                    mt = mrows(t)
                    pd = hA // 2

                    def smm(ps, h, j):
                        off, w = chunks[j]
                        po = (h % 2) * 64
                        nc.tensor.matmul(
                            ps[0:mt, j, 0:w],
                            kt_sb[po:po + 64, pd, 128 * t:128 * t + mt],
                            qt_sb[po:po + 64, pd, off:off + w],
                            start=True, stop=True,
                        )

                    def sexp(ps, e_t, js):
                        flat = ps.rearrange("p a b -> p (a b)")
                        if js is None:
                            nc.scalar.activation(
                                e_t[:, 0:tp], flat[:, 0:tp], AF.Exp,
                                bias=ebias_sb[:, t:t + 1], scale=0.125,
                            )
                            return
                        for j in js:
                            off, w = chunks[j]
                            nc.scalar.activation(
                                e_t[:, off:off + w],
                                flat[:, off:off + w], AF.Exp,
                                bias=ebias_sb[:, t:t + 1], scale=0.125,
                            )

                    if jsplit and nch > 1:
                        st = {}
                        for h in (hA, hB):
                            st[h] = (sps_pool.tile([128, nch, 512], F32,
                                                   tag="s", name="sps"),
                                     get_e(h, t))
                        for h in (hA, hB):
                            ps, e_t = st[h]
                            for j in p1_js:
                                smm(ps, h, j)
                            sexp(ps, e_t, p1_js)

                        def finish():
                            for h in (hA, hB):
                                ps, e_t = st[h]
                                for j in p2_js:
                                    smm(ps, h, j)
                                sexp(ps, e_t, p2_js)

                        return finish
                    for h in (hA, hB):
                        ps = sps_pool.tile([128, nch, 512], F32,
                                           tag="s", name="sps")
                        e_t = get_e(h, t)
                        for j in range(nch):
                            smm(ps, h, j)
                        sexp(ps, e_t, None)
                        if filler:
                            filler()
                    return None

                def av_piece(h, tts, j, first, last, fold=False):
                    # one AV accumulation group: head h, k-tiles tts,
                    # chunk j.  With fold=True (the h2/h3 tail), the
                    # previous acc value is folded in via an identity
                    # matmul so the evacuation is a plain copy that can
                    # run on ACT (idle after the last exp) parallel to DVE.
                    off, w = chunks[j]
                    pool = sps_pool if fold and j < 2 else ops
                    avp = pool.tile([DH + 1, 512], F32,
                                    tag="s" if pool is sps_pool else "o",
                                    name="av")
                    if fold:
                        nc.tensor.matmul(
                            avp[:, 0:w], ident_sb[:],
                            acc_sb[:, h, off:off + w],
                            start=True, stop=False,
                        )
                    for i, t0 in enumerate(tts):
                        st = (i == 0) and not fold
                        sp = (i == len(tts) - 1)
                        mt = mrows(t0)
                        nc.tensor.matmul(
                            avp[:, 0:w],
                            v_sb[0:mt, t0, h, 0:DH + 1],
                            e_tiles[(h, t0)][0:mt, off:off + w],
                            start=st, stop=sp,
                        )
                    if fold:
                        # even head of the pair finishes an exp earlier ->
                        # its evacs go to DVE (free immediately); the odd
                        # head's go to ACT (free right after its last exp)
                        if h % 2 == 1:
                            nc.scalar.activation(
                                acc_sb[:, h, off:off + w], avp[:, 0:w],
                                AF.Identity, bias=0.0)
                        else:
                            nc.vector.tensor_copy(
                                acc_sb[:, h, off:off + w], avp[:, 0:w])
                    elif first:
                        nc.vector.tensor_copy(
                            acc_sb[:, h, off:off + w], avp[:, 0:w])
                    else:
                        nc.vector.tensor_add(
                            acc_sb[:, h, off:off + w],
                            acc_sb[:, h, off:off + w], avp[:, 0:w])
                    if last and not fold:
                        nc.sync.dma_start(
                            out_d.ap()[:, h, off:off + w].bitcast(F32R),
                            acc_sb[:, h, off:off + w])
                    elif last and fold and j == nch - 1:
                        # one whole-head DMA: saves serialized HWDGE slots
                        # on the critical tail
                        nc.sync.dma_start(
                            out_d.ap()[:, h, :].bitcast(F32R),
                            acc_sb[:, h, :])

                # ---- phase 1: h0/h1 scores+exps.  Tile 0 runs j-split:
                # its pass-1-chunk exps seed ACT right after the phase-A
                # evacs, the q-d0-j1 unit runs under them, then the j1
                # continuation.  All v and remaining qk projections
                # drip-feed as filler units under the ACT-paced stream.
                units = deque()
                for off, w in chunks[1:2]:
                    for o2 in range(off, off + w, 256):
                        units.append(("qk", (1, kt_sb, 2, 0, o2,
                                             min(256, off + w - o2))))
                for wi, o_sb, bcol in ((0, qt_sb, 0), (1, kt_sb, 2)):
                    for off, w in chunks:
                        for o2 in range(off, off + w, 256):
                            units.append(
                                ("qk", (wi, o_sb, bcol, 1, o2,
                                        min(256, off + w - o2))))
                for t in proc_t:
                    units.append(("v", t))

                def run_unit(kind, u):
                    if kind == "qk":
                        qk_unit(u)
                    else:
                        v_unit(u)

                def filler():
                    if units:
                        run_unit(*units.popleft())

                fin = scores_pair(0, 1, 0, jsplit=True)
                for off, w in chunks[1:2]:
                    for o2 in range(off, off + w, 256):
                        qk_unit((0, qt_sb, 0, 0, o2,
                                 min(256, off + w - o2)))
                if fin:
                    fin()
                for t in range(1, nkt):
                    scores_pair(0, 1, t, filler=filler)

                # ---- phase 2: h2/h3 scores in proc_t order with AV pieces
                # spread across the slots.  h0/h1 run one accumulation
                # round each (everything is ready at phase start); h2/h3
                # rounds are release-gated and only the small folded last
                # round trails the final exp.
                q01 = deque()
                for h in (0, 1):
                    for j in range(nch):
                        q01.append((h, list(range(nkt)), j, True, True,
                                    False))
                q23 = deque()
                nr23 = len(rounds23)

                def rel23(plo, phi):
                    for gi, (a, b) in enumerate(rounds23):
                        if plo < b <= phi:
                            fold = gi == nr23 - 1 and nr23 > 1
                            for j in range(nch):
                                for h in (2, 3):
                                    q23.append((h, proc_t[a:b], j, gi == 0,
                                                gi == nr23 - 1, fold))

                for p, t in enumerate(proc_t):
                    scores_pair(2, 3, t)
                    rel23(p, p + 1)
                    n = 1 if p < 3 else 2
                    if units:
                        run_unit(*units.popleft())
                        n -= 1
                    while n and (q23 or q01):
                        av_piece(*(q23.popleft() if q23 else q01.popleft()))
                        n -= 1
                while units:
                    run_unit(*units.popleft())
                while q01:
                    av_piece(*q01.popleft())
                rel23(nkt, 10 * nkt)
                while q23:
                    av_piece(*q23.popleft())

    nc.compile()
    return nc


def _get_nc(tp, nkt, ml, with_bv=False):
    key = (tp, nkt, ml, with_bv)
    if key not in _CACHE:
        _CACHE[key] = _build(tp, nkt, ml, with_bv)
    return _CACHE[key]


def _swizzle_w(w, cs):
    """[C, CSL] slice -> [128, 2, NCT, 128] (p, d, i, c) bf16."""
    a = np.ascontiguousarray(w[:, cs:cs + CSL]).reshape(NCT, 128, 2, 128)
    return np.ascontiguousarray(a.transpose(1, 2, 0, 3)).astype(NPFP16)


def kernel(x, Wq, bq, Wk, bk, Wv, bv, mask):
    x = np.asarray(x, dtype=np.float32)
    Wq = np.asarray(Wq, dtype=np.float32)
    bq = np.asarray(bq, dtype=np.float32)
    Wk = np.asarray(Wk, dtype=np.float32)
    bk = np.asarray(bk, dtype=np.float32)
    Wv = np.asarray(Wv, dtype=np.float32)
    bv = np.asarray(bv, dtype=np.float32)
    mask = np.asarray(mask)

    idxs = [np.nonzero(mask[b] != 0)[0] for b in range(B)]
    tvs = [len(ix) for ix in idxs]
    tp, nkt, ml = _pick_dims(max(max(tvs), 1))
    with_bv = bool(np.any(bv))
    nc = _get_nc(tp, nkt, ml, with_bv)

    onesv = np.ones((128, nkt * HPC), NPFP16)

    # per-batch tensors
    xts, ebs = [], []
    for b in range(B):
        xt = np.zeros((C, tp), np.float32)
        if tvs[b]:
            xt[:, :tvs[b]] = x[b][idxs[b]].T
        xts.append(np.ascontiguousarray(
            xt.reshape(NCT, 128, tp).transpose(1, 0, 2)).astype(NPFP16))
        eb = np.full(nkt * 128, -1e30, np.float32)
        eb[:tvs[b]] = EB
        ebs.append(np.ascontiguousarray(eb.reshape(nkt, 128).T))

    in_maps = []
    for core in range(N_CORES):
        b, hg = core // HPC, core % HPC
        cs = hg * CSL
        bias128 = np.concatenate([
            np.stack([bq[cs:cs + 128], bq[cs + 128:cs + 256],
                      bk[cs:cs + 128], bk[cs + 128:cs + 256]], axis=1),
            ebs[b],
        ], axis=1)
        m = {
            "xt": xts[b],
            "wq": _swizzle_w(Wq, cs),
            "wk": _swizzle_w(Wk, cs),
            "wv": np.ascontiguousarray(
                Wv[:, cs:cs + CSL].reshape(NCT, 128, CSL)
                .transpose(1, 0, 2)).astype(NPFP16),
            "bias128": np.ascontiguousarray(bias128),
            "onesv": onesv,
            "ident": np.eye(DH + 1, dtype=np.float32),
        }
        if with_bv:
            m["misc1"] = np.concatenate(
                [bv[cs:cs + CSL], np.ones(128, np.float32)]
            ).reshape(1, -1).astype(NPFP16)
        in_maps.append(m)

    try:
        res = bass_utils.run_bass_kernel_spmd(
            nc, in_maps, core_ids=list(range(N_CORES)), trace=False)
    except Exception:
        # transient axon-worker/NRT failures recover on retry
        res = bass_utils.run_bass_kernel_spmd(
            nc, in_maps, core_ids=list(range(N_CORES)), trace=False)

    y = np.zeros((B, T, C), np.float32)
    for core in range(N_CORES):
        b, hg = core // HPC, core % HPC
        out = res.results[core]["out"]          # [DH+1, HPC, tp]
        ix, tv = idxs[b], tvs[b]
        if not tv:
            continue
        for h in range(HPC):
            numer = out[:DH, h, :tv]
            denom = out[DH, h, :tv]
            col = hg * CSL + h * DH
            y[b, ix, col:col + DH] = (numer / denom).T
    return y
